# revision 1
# baseline (speedup 1.0000x reference)
"""DeepseekV4 indexer (topk_masking) Trainium2 Bass kernel.

Strategy: sequence-parallel over query positions across 8 NeuronCores with an
interleaved row assignment (core c owns rows {c+8k}) so a single SPMD program
is load-balanced under the causal mask. All matmuls run as 3-term fp16 hi/lo
split (hh+hl+lh) accumulating in fp32 PSUM, giving ~1e-6 relative accuracy at
bf16 speed. The relu'd per-head score reduction uses a custom fused DVE op
(S += relu(psum)*w). Top-512 per row via 64 rounds of the DVE max8/
match_replace primitives with causal sentinel values that reproduce
jax.lax.top_k tie ordering.
"""
import sys

for _p in ('/opt/trn_rl_repo',):
    if _p not in sys.path:
        sys.path.insert(0, _p)

import numpy as np
from contextlib import ExitStack

import concourse.bass as bass
from concourse import bacc
import concourse.mybir as mybir
from concourse.tile import TileContext
from concourse import bass_utils
from concourse.masks import make_identity

dt = mybir.dt

B, S, HID = 1, 2048, 2048
H, D, RD, TOPK = 32, 128, 64, 512
ROPE_THETA = 10000.0
NC = 8
ROWS_PER_TILE = 128
EXT_A = 1024   # tile A rows live in [0, 1024): score extent 1024
EXT_B = 2048   # tile B rows live in [1024, 2048)
SENT_BASE = 5.0e4   # sentinel(j) = -(SENT_BASE + j); distinct, below any valid score
CLAMP_AT = -4.5e4   # values below this are sentinels -> clamp to -1e30

# ---------------------------------------------------------------------------
# Custom DVE ops (registered at import; pure-runtime registration)
# ---------------------------------------------------------------------------
_OPS = {}


def _register_custom_ops():
    if _OPS:
        return _OPS
    from concourse import dve_ops as dops
    from concourse.dve_spec import Spec, Src0, Src1, C0, C1, relu, select, lower, Zero, _has_src1
    from concourse.dve_uop import DveOpSpec

    def reg(name, spec, reference):
        for op in dops.OPS:
            if op.name == name:
                _OPS[name] = op
                return
        row = dops._CUSTOM_DVE_ROW_BASE + len(dops.OPS)
        assert row < 0x20, "custom DVE row overflow"
        dops._SUB_OPCODE_FOR_NAME[name] = row
        shas = {}
        for ver in ("v3", "v4"):
            tmp = DveOpSpec(name=name, opcode=row, uops=lower(spec, ver=ver),
                            rd1_en=_has_src1(spec))
            shas[ver] = tmp.sha(ver)
        op = dops.DveOp(name, spec, subdim=False, uops_sha=shas)
        dops.OPS.append(op)
        dops.CUSTOM_DVE_SPECS[name] = spec
        _OPS[name] = op

    # S_acc = relu(in0) * w + S_acc    (w signed per-partition scalar)
    reg("ANT_RELU_WACC",
        Spec(body=relu(Src0) * C0 + Src1,
             reference=lambda in0, in1, s0: np.maximum(in0, 0) * s0 + in1),
        None)
    # out = in0 if jrow <= irow else -(SENT + jrow)
    reg("ANT_CAUSAL_SENT",
        Spec(body=select(Src1 <= C0, Src0, Zero - (Src1 + C1)),
             reference=lambda in0, in1, s0, s1: np.where(in1 <= s0, in0, -(in1 + s1))),
        None)
    # out = in0 if in0 >= c0 else c1
    reg("ANT_CLAMP_SENT",
        Spec(body=select(Src0 >= C0, Src0, C1 + Zero),
             reference=lambda in0, s0, s1: np.where(in0 >= s0, in0, s1)),
        None)
    return _OPS


# ---------------------------------------------------------------------------
# Device program (uniform across cores; per-core variation is data-only)
# ---------------------------------------------------------------------------
_PROGRAM = None


def _f16_pair(x):
    h = x.astype(np.float16)
    l = (x - h.astype(np.float32)).astype(np.float16)
    return h, l


def _build_program():
    global _PROGRAM
    if _PROGRAM is not None:
        return _PROGRAM
    ops = _register_custom_ops()
    RELU_WACC = ops["ANT_RELU_WACC"]
    CAUSAL_SENT = ops["ANT_CAUSAL_SENT"]
    CLAMP_SENT = ops["ANT_CLAMP_SENT"]

    nc = bacc.Bacc("TRN2", target_bir_lowering=False, debug=False, num_devices=NC)

    def din(name, shape, dtype):
        return nc.dram_tensor(name, list(shape), dtype, kind="ExternalInput")

    # replicated inputs
    d_hTh = din("hTh", [HID, S], dt.float16)     # hidden^T fp16 hi   [c, j]
    d_hTl = din("hTl", [HID, S], dt.float16)
    d_wqh = din("wqh", [HID, H * D], dt.float16)
    d_wql = din("wql", [HID, H * D], dt.float16)
    d_wkh = din("wkh", [HID, D], dt.float16)
    d_wkl = din("wkl", [HID, D], dt.float16)
    d_wwh = din("wwh", [HID, H], dt.float16)     # pre-scaled by H^-.5 * D^-.5
    d_wwl = din("wwl", [HID, H], dt.float16)
    d_c2T = din("cos2T", [RD, S], dt.float32)    # expanded cos, transposed
    d_s2T = din("sin2T", [RD, S], dt.float32)
    d_MT = din("MT", [D, D], dt.float32)         # rope rotation matrix (lhsT form)
    d_jrow = din("jrow", [1, S], dt.float32)     # iota row 0..S-1
    # per-core inputs
    d_ohTh = din("ohTh", [HID, 2 * ROWS_PER_TILE], dt.float16)  # own rows^T (A|B)
    d_ohTl = din("ohTl", [HID, 2 * ROWS_PER_TILE], dt.float16)
    d_cosA = din("cosA", [ROWS_PER_TILE, RD // 2], dt.float32)  # half tables
    d_sinA = din("sinA", [ROWS_PER_TILE, RD // 2], dt.float32)
    d_cosB = din("cosB", [ROWS_PER_TILE, RD // 2], dt.float32)
    d_sinB = din("sinB", [ROWS_PER_TILE, RD // 2], dt.float32)
    d_irowA = din("irowA", [ROWS_PER_TILE, 1], dt.float32)      # global row idx
    d_irowB = din("irowB", [ROWS_PER_TILE, 1], dt.float32)

    outs = {}
    for t in ("A", "B"):
        outs[f"oV{t}"] = nc.dram_tensor(f"oV{t}", [ROWS_PER_TILE, TOPK], dt.float32,
                                        kind="ExternalOutput")
        outs[f"oI{t}"] = nc.dram_tensor(f"oI{t}", [ROWS_PER_TILE, TOPK], dt.uint32,
                                        kind="ExternalOutput")

    NCHUNK = HID // 128  # 16 contraction chunks

    with TileContext(nc) as tc, ExitStack() as ctx:
        const = ctx.enter_context(tc.tile_pool(name="const", bufs=1))
        sb = ctx.enter_context(tc.tile_pool(name="sb", bufs=1))
        stream = ctx.enter_context(tc.tile_pool(name="stream", bufs=2))

        # ---- small constants in SBUF ----
        t_ohTh = const.tile([128, NCHUNK * 256], dt.float16)
        t_ohTl = const.tile([128, NCHUNK * 256], dt.float16)
        for c in range(NCHUNK):
            nc.sync.dma_start(t_ohTh[:, c * 256:(c + 1) * 256], d_ohTh.ap()[c * 128:(c + 1) * 128, :])
            nc.sync.dma_start(t_ohTl[:, c * 256:(c + 1) * 256], d_ohTl.ap()[c * 128:(c + 1) * 128, :])
        t_c2T_f = const.tile([128, S], dt.float32, name="t_c2T_f")
        t_c2T = t_c2T_f[D - RD:, :]
        nc.sync.dma_start(t_c2T, d_c2T.ap())
        t_s2T_f = const.tile([128, S], dt.float32, name="t_s2T_f")
        t_s2T = t_s2T_f[D - RD:, :]
        nc.sync.dma_start(t_s2T, d_s2T.ap())
        t_MT = const.tile([D, D], dt.float32)
        nc.sync.dma_start(t_MT[:], d_MT.ap())
        t_jrow = const.tile([128, S], dt.float32)
        nc.sync.dma_start(t_jrow[:], d_jrow.ap().to_broadcast([128, S]))
        t_cos = {}
        for nm, dte in (("cosA", d_cosA), ("sinA", d_sinA), ("cosB", d_cosB), ("sinB", d_sinB)):
            t_cos[nm] = const.tile([ROWS_PER_TILE, RD // 2], dt.float32, name=f"t_{nm}")
            nc.sync.dma_start(t_cos[nm][:], dte.ap())
        t_irow = {}
        for nm, dte in (("A", d_irowA), ("B", d_irowB)):
            t_irow[nm] = const.tile([ROWS_PER_TILE, 1], dt.float32, name=f"t_irow{nm}")
            nc.sync.dma_start(t_irow[nm][:], dte.ap())
        ident16 = const.tile([128, 128], dt.float16)
        make_identity(nc, ident16[:])

        # =========== Phase K: kT projection + rope + fp16 split =============
        t_kT = sb.tile([D, S], dt.float32, tag="kTf32")
        with tc.tile_pool(name="psk", bufs=1, space="PSUM") as psk:
            ps_kT = psk.tile([D, S], dt.float32, tag="pskT")
            for c in range(NCHUNK):
                kh = stream.tile([128, NCHUNK * 512], dt.float16, tag="wqh", name="kh")[:, :S]
                nc.sync.dma_start(kh[:], d_hTh.ap()[c * 128:(c + 1) * 128, :])
                kl = stream.tile([128, NCHUNK * 512], dt.float16, tag="wql", name="kl")[:, :S]
                nc.sync.dma_start(kl[:], d_hTl.ap()[c * 128:(c + 1) * 128, :])
                wkh_c = stream.tile([128, D], dt.float16, tag="wkh")
                nc.sync.dma_start(wkh_c[:], d_wkh.ap()[c * 128:(c + 1) * 128, :])
                wkl_c = stream.tile([128, D], dt.float16, tag="wkl")
                nc.sync.dma_start(wkl_c[:], d_wkl.ap()[c * 128:(c + 1) * 128, :])
                first = (c == 0)
                last = (c == NCHUNK - 1)
                for jb in range(S // 512):
                    sl = slice(jb * 512, (jb + 1) * 512)
                    nc.tensor.matmul(ps_kT[:, sl], wkh_c[:], kh[:, sl], start=first, stop=False)
                    nc.tensor.matmul(ps_kT[:, sl], wkh_c[:], kl[:, sl], start=False, stop=False)
                    nc.tensor.matmul(ps_kT[:, sl], wkl_c[:], kh[:, sl], start=False, stop=last)
            for jb in range(S // 512):
                sl = slice(jb * 512, (jb + 1) * 512)
                nc.scalar.copy(t_kT[:, sl], ps_kT[:, sl])

            # rope on kT: rot = MT.T @ kT (rows 64.. hold the pair-swapped rope dims)
            ps_rot = psk.tile([D, S], dt.float32, tag="pskT")
            for jb in range(S // 512):
                sl = slice(jb * 512, (jb + 1) * 512)
                nc.tensor.matmul(ps_rot[:, sl], t_MT[:], t_kT[:, sl], start=True, stop=True)
            t_rot_f = sb.tile([128, S], dt.float32, tag="rotk", name="t_rot_f")
            t_rot = t_rot_f[D - RD:, :]
            for jb in range(S // 512):
                sl = slice(jb * 512, (jb + 1) * 512)
                nc.scalar.copy(t_rot[:, sl], ps_rot[D - RD:, sl])
        # krope = kT[64:]*cos2T + rot*sin2T   (all on partitions 64..127)
        nc.vector.tensor_mul(t_rot, t_rot, t_s2T)
        t_kr2_f = sb.tile([128, S], dt.float32, tag="kr2", name="t_kr2_f")
        t_krope = t_kr2_f[D - RD:, :]
        nc.vector.tensor_mul(t_krope, t_kT[D - RD:, :], t_c2T)
        nc.vector.tensor_add(t_krope, t_rot, t_krope)
        # split to fp16 pair
        t_kTh = sb.tile([D, S], dt.float16, tag="kTh")
        t_kTl = sb.tile([D, S], dt.float16, tag="kTl")
        nc.vector.tensor_copy(t_kTh[:D - RD, :], t_kT[:D - RD, :])
        nc.vector.tensor_copy(t_kTh[D - RD:, :], t_krope)
        nc.vector.tensor_sub(t_kTl[:D - RD, :], t_kT[:D - RD, :], t_kTh[:D - RD, :])
        nc.vector.tensor_sub(t_kTl[D - RD:, :], t_krope, t_kTh[D - RD:, :])

        # =========== Phase Q: q/w projection, rope, split, transpose ========
        t_w = {}
        rqT = {t: (sb.tile([128, H * D], dt.float16, tag=f"rqTh{t}", name=f"rqTh{t}"),
                   sb.tile([128, H * D], dt.float16, tag=f"rqTl{t}", name=f"rqTl{t}"))
               for t in ("A", "B")}
        EBG = 512
        HPG = EBG // D  # heads per ebg group
        with tc.tile_pool(name="psq", bufs=2, space="PSUM") as psq_pool, \
             tc.tile_pool(name="psw", bufs=1, space="PSUM") as psw_pool:
            ps_w = {t: psw_pool.tile([128, H], dt.float32, tag=f"psw{t}", name=f"psw{t}") for t in ("A", "B")}
            wwpack = const.tile([128, 2 * NCHUNK * H], dt.float16)
            for c in range(NCHUNK):
                nc.sync.dma_start(wwpack[:, c * H:(c + 1) * H], d_wwh.ap()[c * 128:(c + 1) * 128, :])
                nc.sync.dma_start(wwpack[:, NCHUNK * H + c * H:NCHUNK * H + (c + 1) * H],
                                  d_wwl.ap()[c * 128:(c + 1) * 128, :])
            wwh_s = wwpack[:, :NCHUNK * H]
            wwl_s = wwpack[:, NCHUNK * H:]
            for ebg in range(H * D // EBG):
                esl = slice(ebg * EBG, (ebg + 1) * EBG)
                wqh_s = stream.tile([128, NCHUNK * EBG], dt.float16, tag="wqh")
                wql_s = stream.tile([128, NCHUNK * EBG], dt.float16, tag="wql")
                for c in range(NCHUNK):
                    nc.sync.dma_start(wqh_s[:, c * EBG:(c + 1) * EBG], d_wqh.ap()[c * 128:(c + 1) * 128, esl])
                    nc.sync.dma_start(wql_s[:, c * EBG:(c + 1) * EBG], d_wql.ap()[c * 128:(c + 1) * 128, esl])
                for ti, t in enumerate(("A", "B")):
                    ps_q = psq_pool.tile([128, EBG], dt.float32, tag="psq")
                    for c in range(NCHUNK):
                        base = c * 256 + ti * 128
                        lhs_h = t_ohTh[:, base:base + 128]
                        lhs_l = t_ohTl[:, base:base + 128]
                        wq_h = wqh_s[:, c * EBG:(c + 1) * EBG]
                        wq_l = wql_s[:, c * EBG:(c + 1) * EBG]
                        first = (c == 0)
                        last = (c == NCHUNK - 1)
                        nc.tensor.matmul(ps_q[:], lhs_h, wq_h, start=first, stop=False)
                        nc.tensor.matmul(ps_q[:], lhs_h, wq_l, start=False, stop=False)
                        if ebg == 0:
                            nc.tensor.matmul(ps_w[t][:], lhs_h, wwh_s[:, c * H:(c + 1) * H],
                                             start=first, stop=False)
                            nc.tensor.matmul(ps_w[t][:], lhs_h, wwl_s[:, c * H:(c + 1) * H],
                                             start=False, stop=False)
                            nc.tensor.matmul(ps_w[t][:], lhs_l, wwh_s[:, c * H:(c + 1) * H],
                                             start=False, stop=False)
                            nc.tensor.matmul(ps_w[t][:], lhs_l, wwl_s[:, c * H:(c + 1) * H],
                                             start=False, stop=last)
                        nc.tensor.matmul(ps_q[:], lhs_l, wq_h, start=False, stop=last)
                    # evict this 4-head group, rope it, split, transpose
                    q32s = sb.tile([128, EBG], dt.float32, tag="q32", name=f"q32{t}{ebg}")
                    nc.scalar.copy(q32s[:], ps_q[:])
                    if ebg == 0:
                        t_w[t] = sb.tile([128, H], dt.float32, tag=f"w{t}", name=f"tw{t}")
                        nc.vector.tensor_scalar_mul(t_w[t][:], ps_w[t][:],
                                                    float((H * D) ** -0.5))
                    cosb = t_cos["cos" + t][:].rearrange("p (x m) -> p x m", x=1).to_broadcast([128, HPG, RD // 2])
                    sinb = t_cos["sin" + t][:].rearrange("p (x m) -> p x m", x=1).to_broadcast([128, HPG, RD // 2])
                    qv = q32s[:].rearrange("p (h d) -> p h d", h=HPG)
                    viewE = qv[:, :, D - RD::2]     # [128, HPG, 32] even rope cols
                    viewO = qv[:, :, D - RD + 1::2]
                    tmp = [sb.tile([128, HPG * (RD // 2)], dt.float32, tag=f"ropetmp{k}",
                                   name=f"ropetmp{t}{ebg}_{k}")
                           for k in range(4)]
                    tv = [x[:].rearrange("p (h m) -> p h m", h=HPG) for x in tmp]
                    nc.vector.tensor_mul(tv[0], viewO, sinb)  # tE
                    nc.vector.tensor_mul(tv[1], viewE, sinb)  # tO
                    nc.vector.tensor_mul(tv[2], viewE, cosb)  # m1
                    nc.vector.tensor_mul(tv[3], viewO, cosb)  # m2
                    nc.vector.tensor_sub(viewE, tv[2], tv[0])
                    nc.vector.tensor_add(viewO, tv[3], tv[1])
                    # split to fp16 pair
                    qh = sb.tile([128, EBG], dt.float16, tag="qh", name=f"qh{t}{ebg}")
                    ql = sb.tile([128, EBG], dt.float16, tag="ql", name=f"ql{t}{ebg}")
                    nc.vector.tensor_copy(qh[:], q32s[:])
                    nc.vector.tensor_sub(ql[:], q32s[:], qh[:])
                    # transpose 4 heads -> rqT [d, i] slices
                    for src, dst in ((qh, rqT[t][0]), (ql, rqT[t][1])):
                        ps_t = psq_pool.tile([128, EBG], dt.float16, tag="pstr",
                                             name=f"pstr{t}{ebg}")
                        for hh in range(HPG):
                            nc.tensor.transpose(ps_t[:, hh * D:(hh + 1) * D],
                                                src[:, hh * D:(hh + 1) * D], ident16[:])
                        nc.scalar.copy(dst[:, esl], ps_t[:])

        # =========== Phase S: scores + fused relu*w head reduction ==========
        # =========== Phase T: causal sentinel mask + topk ===================
        pss_pool = ctx.enter_context(tc.tile_pool(name="pss", bufs=2, space="PSUM"))
        Sacc, Smask, vals, idxs = {}, {}, {}, {}
        EXTS = {"A": EXT_A, "B": EXT_B}
        for t, EXT in (("A", EXT_A), ("B", EXT_B)):
            rqTh, rqTl = rqT[t]
            Sacc[t] = sb.tile([128, EXT], dt.float32, tag=f"sacc{t}", name=f"Sacc{t}")
            nc.vector.memset(Sacc[t][:], 0.0)
            for h in range(H):
                ps_s = pss_pool.tile([128, EXT], dt.float32, tag="pss")
                lh = rqTh[:, h * D:(h + 1) * D]
                ll = rqTl[:, h * D:(h + 1) * D]
                for jb in range(EXT // 512):
                    sl = slice(jb * 512, (jb + 1) * 512)
                    nc.tensor.matmul(ps_s[:, sl], lh, t_kTh[:, sl], start=True, stop=False)
                    nc.tensor.matmul(ps_s[:, sl], lh, t_kTl[:, sl], start=False, stop=False)
                    nc.tensor.matmul(ps_s[:, sl], ll, t_kTh[:, sl], start=False, stop=True)
                nc.vector._custom_dve(_OPS["ANT_RELU_WACC"], out=Sacc[t][:], in0=ps_s[:],
                                      in1=Sacc[t][:], s0=t_w[t][:, h:h + 1])
            # causal mask + sentinels: Smask = masked scores (kept for max_index),
            # Sacc stays the destructive work array for the rounds
            Smask[t] = sb.tile([128, EXT], dt.float32, tag="rotk" if t == "A" else "kr2", name=f"Smask{t}")
            nc.vector._custom_dve(_OPS["ANT_CAUSAL_SENT"], out=Smask[t][:], in0=Sacc[t][:],
                                  in1=t_jrow[:, :EXT],
                                  s0=t_irow[t][:], s1=SENT_BASE)
            nc.vector.tensor_copy(Sacc[t][:], Smask[t][:])
            vals[t] = sb.tile([128, TOPK], dt.float32, tag=f"vals{t}", name=f"vals{t}")
            idxs[t] = sb.tile([128, TOPK], dt.uint32, tag=f"idx{t}", name=f"idx{t}")
            # per-tile topk emitted right away: tile A's DVE chain becomes
            # schedulable during tile B's score matmuls
            for r in range(TOPK // 8):
                v8 = vals[t][:, r * 8:(r + 1) * 8]
                nc.vector.max(out=v8, in_=Sacc[t][:])
                nc.vector.match_replace(out=Sacc[t][:], in_to_replace=v8,
                                        in_values=Sacc[t][:], imm_value=-3.0e38)
            for r in range(TOPK // 8):
                nc.vector.max_index(out=idxs[t][:, r * 8:(r + 1) * 8],
                                    in_max=vals[t][:, r * 8:(r + 1) * 8],
                                    in_values=Smask[t][:])
            cl = sb.tile([128, TOPK], dt.float32, tag="cl", name=f"cl{t}")
            nc.vector._custom_dve(_OPS["ANT_CLAMP_SENT"], out=cl[:], in0=vals[t][:],
                                  s0=CLAMP_AT, s1=-1.0e30)
            nc.sync.dma_start(outs[f"oV{t}"].ap(), cl[:])
            nc.sync.dma_start(outs[f"oI{t}"].ap(), idxs[t][:])

    nc.compile()
    _PROGRAM = nc
    return nc


# ---------------------------------------------------------------------------
# Host wrapper
# ---------------------------------------------------------------------------

def _host_inputs(hidden_states, cos, sin, wq, wk, ww):
    hid = hidden_states.reshape(S, HID).astype(np.float32)
    hT = np.ascontiguousarray(hid.T)
    hTh, hTl = _f16_pair(hT)
    wqh, wql = _f16_pair(wq.astype(np.float32))
    wkh, wkl = _f16_pair(wk.astype(np.float32))
    wwh, wwl = _f16_pair(ww.astype(np.float32))
    cosf = cos.reshape(S, RD // 2).astype(np.float32)
    sinf = sin.reshape(S, RD // 2).astype(np.float32)
    cos2 = np.repeat(cosf, 2, axis=1)            # [S, RD]
    sin2 = np.repeat(sinf, 2, axis=1)
    cos2T = np.ascontiguousarray(cos2.T)         # [RD, S]
    sin2T = np.ascontiguousarray(sin2.T)
    # rope rotation matrix: rot = M @ kvec on the last RD dims;
    # matmul computes lhsT.T @ rhs -> lhsT = M.T
    M = np.zeros((D, D), dtype=np.float32)
    for m in range(RD // 2):
        e = D - RD + 2 * m
        M[e, e + 1] = -1.0
        M[e + 1, e] = 1.0
    MT = np.ascontiguousarray(M.T)
    jrow = np.arange(S, dtype=np.float32).reshape(1, S)

    rep = {"hTh": hTh, "hTl": hTl, "wqh": wqh, "wql": wql, "wkh": wkh,
           "wkl": wkl, "wwh": wwh, "wwl": wwl, "cos2T": cos2T, "sin2T": sin2T,
           "MT": MT, "jrow": jrow}

    in_maps, row_maps = [], []
    for c in range(NC):
        rowsA = np.arange(c, EXT_A, NC, dtype=np.int64)
        rowsB = np.arange(EXT_A + c, S, NC, dtype=np.int64)
        own = np.concatenate([rowsA, rowsB])
        ohT = np.ascontiguousarray(hT[:, own])
        ohTh, ohTl = _f16_pair(ohT)
        m = dict(rep)
        m["ohTh"] = ohTh
        m["ohTl"] = ohTl
        m["cosA"] = np.ascontiguousarray(cosf[rowsA])
        m["sinA"] = np.ascontiguousarray(sinf[rowsA])
        m["cosB"] = np.ascontiguousarray(cosf[rowsB])
        m["sinB"] = np.ascontiguousarray(sinf[rowsB])
        m["irowA"] = rowsA.astype(np.float32).reshape(-1, 1)
        m["irowB"] = rowsB.astype(np.float32).reshape(-1, 1)
        in_maps.append(m)
        row_maps.append((rowsA, rowsB))
    return in_maps, row_maps


def kernel(hidden_states, cos, sin, wq, wk, ww, _trace=False):
    hidden_states = np.asarray(hidden_states)
    nc = _build_program()
    in_maps, row_maps = _host_inputs(np.asarray(hidden_states), np.asarray(cos),
                                     np.asarray(sin), np.asarray(wq), np.asarray(wk),
                                     np.asarray(ww))
    res = bass_utils.run_bass_kernel_spmd(nc, in_maps, core_ids=list(range(NC)),
                                          trace=_trace)
    scores = np.zeros((B, S, TOPK), dtype=np.float32)
    idxs = np.zeros((B, S, TOPK), dtype=np.int32)
    for c in range(NC):
        rowsA, rowsB = row_maps[c]
        r = res.results[c]
        scores[0, rowsA] = r["oVA"]
        scores[0, rowsB] = r["oVB"]
        idxs[0, rowsA] = r["oIA"].astype(np.int32)
        idxs[0, rowsB] = r["oIB"].astype(np.int32)
    kernel._last_result = res
    return scores, idxs



# revision 2
# speedup vs baseline: 1.2369x; 1.2369x over previous
"""DeepseekV4 indexer (topk_masking) Trainium2 Bass kernel.

Sequence-parallel over query positions across 8 NeuronCores. Core c owns two
CONTIGUOUS 128-row blocks: big block 15-c (rows [128*(15-c), 128*(16-c)),
causal extent e_B = 128*(16-c)) and small block c (extent e_S = 128*(c+1)).
e_B + e_S = 2176 for every core, vs 3072 for the uniform interleaved split --
a 30% cut in both score-matmul and top-k DVE work. Per-core extents are
compile-time distinct, dispatched via tc.Switch on the runtime core id.

All matmuls run as 3-term fp16 hi/lo split (hh+hl+lh) accumulating in fp32
PSUM (~1e-6 relative accuracy; required -- top-k index order flips scale
linearly with score error and the harness error budget is nearly consumed
by flips already at 1e-6). Top-512 per row via rounds of DVE max8 /
find_index8 / match_replace8 on a single working array, with causal sentinel
values that reproduce jax.lax.top_k tie ordering. Rows with extent < 512 get
their deterministic sentinel tail filled on the host.
"""
import sys

for _p in ('/opt/trn_rl_repo',):
    if _p not in sys.path:
        sys.path.insert(0, _p)

import numpy as np
from contextlib import ExitStack

import concourse.bass as bass
from concourse import bacc
import concourse.mybir as mybir
from concourse.tile import TileContext
from concourse import bass_utils
from concourse.masks import make_identity

dt = mybir.dt
ET = mybir.EngineType

B, S, HID = 1, 2048, 2048
H, D, RD, TOPK = 32, 128, 64, 512
NC = 8
SENT_BASE = 5.0e4   # sentinel(j) = -(SENT_BASE + j); distinct, below any valid score
CLAMP_AT = -4.5e4   # values below this are sentinels -> clamp to -1e30

# per-core block assignment: (big, small) = (15-c, c)
BLK_BIG = [15 - c for c in range(NC)]
BLK_SML = [c for c in range(NC)]


def _ext(b):
    return 128 * (b + 1)


def _rounds(e):
    # extraction rounds: 8 ranks per round; ranks beyond min(e,512) are
    # deterministic sentinels (host-filled)
    return (min(e, TOPK) + 7) // 8


# ---------------------------------------------------------------------------
# Custom DVE ops (registered at import; pure-runtime registration)
# ---------------------------------------------------------------------------
_OPS = {}


def _register_custom_ops():
    if _OPS:
        return _OPS
    from concourse import dve_ops as dops
    from concourse.dve_spec import Spec, Src0, Src1, C0, C1, relu, select, lower, Zero, _has_src1
    from concourse.dve_uop import DveOpSpec

    def reg(name, spec):
        for op in dops.OPS:
            if op.name == name:
                _OPS[name] = op
                return
        row = dops._CUSTOM_DVE_ROW_BASE + len(dops.OPS)
        assert row < 0x20, "custom DVE row overflow"
        dops._SUB_OPCODE_FOR_NAME[name] = row
        shas = {}
        for ver in ("v3", "v4"):
            tmp = DveOpSpec(name=name, opcode=row, uops=lower(spec, ver=ver),
                            rd1_en=_has_src1(spec))
            shas[ver] = tmp.sha(ver)
        op = dops.DveOp(name, spec, subdim=False, uops_sha=shas)
        dops.OPS.append(op)
        dops.CUSTOM_DVE_SPECS[name] = spec
        _OPS[name] = op

    # S_acc = relu(in0) * w + S_acc    (w signed per-partition scalar)
    reg("ANT_RELU_WACC",
        Spec(body=relu(Src0) * C0 + Src1,
             reference=lambda in0, in1, s0: np.maximum(in0, 0) * s0 + in1))
    # out = in0 if jrow <= irow else -(SENT + jrow)
    reg("ANT_CAUSAL_SENT",
        Spec(body=select(Src1 <= C0, Src0, Zero - (Src1 + C1)),
             reference=lambda in0, in1, s0, s1: np.where(in1 <= s0, in0, -(in1 + s1))))
    # out = in0 if in0 >= c0 else c1
    reg("ANT_CLAMP_SENT",
        Spec(body=select(Src0 >= C0, Src0, C1 + Zero),
             reference=lambda in0, s0, s1: np.where(in0 >= s0, in0, s1)))
    return _OPS


# ---------------------------------------------------------------------------
# Device program
# ---------------------------------------------------------------------------
_PROGRAM = None


def _f16_pair(x):
    h = x.astype(np.float16)
    l = (x - h.astype(np.float32)).astype(np.float16)
    return h, l


def _build_program():
    global _PROGRAM
    if _PROGRAM is not None:
        return _PROGRAM
    ops = _register_custom_ops()

    nc = bacc.Bacc("TRN2", target_bir_lowering=False, debug=False, num_devices=NC)

    def din(name, shape, dtype):
        return nc.dram_tensor(name, list(shape), dtype, kind="ExternalInput")

    # replicated inputs
    d_hTh = din("hTh", [HID, S], dt.float16)     # hidden^T fp16 hi   [c, j]
    d_hTl = din("hTl", [HID, S], dt.float16)
    d_wqh = din("wqh", [HID, H * D], dt.float16)
    d_wql = din("wql", [HID, H * D], dt.float16)
    d_wkh = din("wkh", [HID, D], dt.float16)
    d_wkl = din("wkl", [HID, D], dt.float16)
    d_wwh = din("wwh", [HID, H], dt.float16)     # pre-scaled by H^-.5 * D^-.5
    d_wwl = din("wwl", [HID, H], dt.float16)
    d_c2T = din("cos2T", [RD, S], dt.float32)    # expanded cos, transposed
    d_s2T = din("sin2T", [RD, S], dt.float32)
    d_MT = din("MT", [D, D], dt.float32)         # rope rotation matrix (lhsT form)
    d_jrow = din("jrow", [1, S], dt.float32)     # iota row 0..S-1
    # per-core inputs: big-block rows first, then small-block rows
    d_ohTh = din("ohTh", [HID, 256], dt.float16)
    d_ohTl = din("ohTl", [HID, 256], dt.float16)
    d_cosB = din("cosB", [128, RD // 2], dt.float32)
    d_sinB = din("sinB", [128, RD // 2], dt.float32)
    d_cosS = din("cosS", [128, RD // 2], dt.float32)
    d_sinS = din("sinS", [128, RD // 2], dt.float32)
    d_irowB = din("irowB", [128, 1], dt.float32)
    d_irowS = din("irowS", [128, 1], dt.float32)

    outs = {}
    for t in ("B", "S"):
        outs[f"oV{t}"] = nc.dram_tensor(f"oV{t}", [128, TOPK], dt.float32,
                                        kind="ExternalOutput")
        outs[f"oI{t}"] = nc.dram_tensor(f"oI{t}", [128, TOPK], dt.uint32,
                                        kind="ExternalOutput")

    NCHUNK = HID // 128  # 16 contraction chunks

    with TileContext(nc) as tc, ExitStack() as ctx:
        const = ctx.enter_context(tc.tile_pool(name="const", bufs=1))
        sb = ctx.enter_context(tc.tile_pool(name="sb", bufs=1))
        stream = ctx.enter_context(tc.tile_pool(name="stream", bufs=2))

        # ---- small constants in SBUF ----
        t_ohTh = const.tile([128, NCHUNK * 256], dt.float16)
        t_ohTl = const.tile([128, NCHUNK * 256], dt.float16)
        for c in range(NCHUNK):
            nc.sync.dma_start(t_ohTh[:, c * 256:(c + 1) * 256], d_ohTh.ap()[c * 128:(c + 1) * 128, :])
            nc.sync.dma_start(t_ohTl[:, c * 256:(c + 1) * 256], d_ohTl.ap()[c * 128:(c + 1) * 128, :])
        t_c2T_f = const.tile([128, S], dt.float32, name="t_c2T_f")
        t_c2T = t_c2T_f[D - RD:, :]
        nc.sync.dma_start(t_c2T, d_c2T.ap())
        t_s2T_f = const.tile([128, S], dt.float32, name="t_s2T_f")
        t_s2T = t_s2T_f[D - RD:, :]
        nc.sync.dma_start(t_s2T, d_s2T.ap())
        t_MT = const.tile([D, D], dt.float32)
        nc.sync.dma_start(t_MT[:], d_MT.ap())
        t_jrow = const.tile([128, S], dt.float32)
        nc.sync.dma_start(t_jrow[:], d_jrow.ap().to_broadcast([128, S]))
        t_cos = {}
        for nm, dte in (("cosB", d_cosB), ("sinB", d_sinB), ("cosS", d_cosS), ("sinS", d_sinS)):
            t_cos[nm] = const.tile([128, RD // 2], dt.float32, name=f"t_{nm}")
            nc.sync.dma_start(t_cos[nm][:], dte.ap())
        t_irow = {}
        for nm, dte in (("B", d_irowB), ("S", d_irowS)):
            t_irow[nm] = const.tile([128, 1], dt.float32, name=f"t_irow{nm}")
            nc.sync.dma_start(t_irow[nm][:], dte.ap())
        ident16 = const.tile([128, 128], dt.float16)
        make_identity(nc, ident16[:])

        # =========== Phase K: kT projection + rope + fp16 split =============
        t_kT = sb.tile([D, S], dt.float32, tag="kTf32")
        with tc.tile_pool(name="psk", bufs=1, space="PSUM") as psk:
            ps_kT = psk.tile([D, S], dt.float32, tag="pskT")
            for c in range(NCHUNK):
                kh = stream.tile([128, NCHUNK * 512], dt.float16, tag="wqh", name="kh")[:, :S]
                nc.sync.dma_start(kh[:], d_hTh.ap()[c * 128:(c + 1) * 128, :])
                kl = stream.tile([128, NCHUNK * 512], dt.float16, tag="wql", name="kl")[:, :S]
                nc.sync.dma_start(kl[:], d_hTl.ap()[c * 128:(c + 1) * 128, :])
                wkh_c = stream.tile([128, D], dt.float16, tag="wkh")
                nc.sync.dma_start(wkh_c[:], d_wkh.ap()[c * 128:(c + 1) * 128, :])
                wkl_c = stream.tile([128, D], dt.float16, tag="wkl")
                nc.sync.dma_start(wkl_c[:], d_wkl.ap()[c * 128:(c + 1) * 128, :])
                first = (c == 0)
                last = (c == NCHUNK - 1)
                for jb in range(S // 512):
                    sl = slice(jb * 512, (jb + 1) * 512)
                    nc.tensor.matmul(ps_kT[:, sl], wkh_c[:], kh[:, sl], start=first, stop=False)
                    nc.tensor.matmul(ps_kT[:, sl], wkh_c[:], kl[:, sl], start=False, stop=False)
                    nc.tensor.matmul(ps_kT[:, sl], wkl_c[:], kh[:, sl], start=False, stop=last)
            for jb in range(S // 512):
                sl = slice(jb * 512, (jb + 1) * 512)
                nc.scalar.copy(t_kT[:, sl], ps_kT[:, sl])

            # rope on kT: rot = MT.T @ kT (rows 64.. hold the pair-swapped rope dims)
            ps_rot = psk.tile([D, S], dt.float32, tag="pskT")
            for jb in range(S // 512):
                sl = slice(jb * 512, (jb + 1) * 512)
                nc.tensor.matmul(ps_rot[:, sl], t_MT[:], t_kT[:, sl], start=True, stop=True)
            t_rot_f = sb.tile([128, S], dt.float32, tag="rotk", name="t_rot_f")
            t_rot = t_rot_f[D - RD:, :]
            for jb in range(S // 512):
                sl = slice(jb * 512, (jb + 1) * 512)
                nc.scalar.copy(t_rot[:, sl], ps_rot[D - RD:, sl])
        # krope = kT[64:]*cos2T + rot*sin2T   (all on partitions 64..127)
        nc.vector.tensor_mul(t_rot, t_rot, t_s2T)
        t_kr2_f = sb.tile([128, S], dt.float32, tag="kr2", name="t_kr2_f")
        t_krope = t_kr2_f[D - RD:, :]
        nc.vector.tensor_mul(t_krope, t_kT[D - RD:, :], t_c2T)
        nc.vector.tensor_add(t_krope, t_rot, t_krope)
        # split to fp16 pair
        t_kTh = sb.tile([D, S], dt.float16, tag="kTh")
        t_kTl = sb.tile([D, S], dt.float16, tag="kTl")
        nc.vector.tensor_copy(t_kTh[:D - RD, :], t_kT[:D - RD, :])
        nc.vector.tensor_copy(t_kTh[D - RD:, :], t_krope)
        nc.vector.tensor_sub(t_kTl[:D - RD, :], t_kT[:D - RD, :], t_kTh[:D - RD, :])
        nc.vector.tensor_sub(t_kTl[D - RD:, :], t_krope, t_kTh[D - RD:, :])

        # =========== Phase Q: q/w projection, rope, split, transpose ========
        # tile "B" (big block) first: its score matmuls and DVE chain are the
        # longest, so its q must be ready earliest.
        t_w = {}
        rqT = {t: (sb.tile([128, H * D], dt.float16, tag=f"rqTh{t}", name=f"rqTh{t}"),
                   sb.tile([128, H * D], dt.float16, tag=f"rqTl{t}", name=f"rqTl{t}"))
               for t in ("B", "S")}
        EBG = 512
        HPG = EBG // D  # heads per ebg group
        with tc.tile_pool(name="psq", bufs=2, space="PSUM") as psq_pool, \
             tc.tile_pool(name="psw", bufs=1, space="PSUM") as psw_pool:
            ps_w = {t: psw_pool.tile([128, H], dt.float32, tag=f"psw{t}", name=f"psw{t}") for t in ("B", "S")}
            wwpack = const.tile([128, 2 * NCHUNK * H], dt.float16)
            for c in range(NCHUNK):
                nc.sync.dma_start(wwpack[:, c * H:(c + 1) * H], d_wwh.ap()[c * 128:(c + 1) * 128, :])
                nc.sync.dma_start(wwpack[:, NCHUNK * H + c * H:NCHUNK * H + (c + 1) * H],
                                  d_wwl.ap()[c * 128:(c + 1) * 128, :])
            wwh_s = wwpack[:, :NCHUNK * H]
            wwl_s = wwpack[:, NCHUNK * H:]
            for ebg in range(H * D // EBG):
                esl = slice(ebg * EBG, (ebg + 1) * EBG)
                wqh_s = stream.tile([128, NCHUNK * EBG], dt.float16, tag="wqh")
                wql_s = stream.tile([128, NCHUNK * EBG], dt.float16, tag="wql")
                for c in range(NCHUNK):
                    nc.sync.dma_start(wqh_s[:, c * EBG:(c + 1) * EBG], d_wqh.ap()[c * 128:(c + 1) * 128, esl])
                    nc.sync.dma_start(wql_s[:, c * EBG:(c + 1) * EBG], d_wql.ap()[c * 128:(c + 1) * 128, esl])
                for ti, t in enumerate(("B", "S")):
                    ps_q = psq_pool.tile([128, EBG], dt.float32, tag="psq")
                    for c in range(NCHUNK):
                        base = c * 256 + ti * 128
                        lhs_h = t_ohTh[:, base:base + 128]
                        lhs_l = t_ohTl[:, base:base + 128]
                        wq_h = wqh_s[:, c * EBG:(c + 1) * EBG]
                        wq_l = wql_s[:, c * EBG:(c + 1) * EBG]
                        first = (c == 0)
                        last = (c == NCHUNK - 1)
                        nc.tensor.matmul(ps_q[:], lhs_h, wq_h, start=first, stop=False)
                        nc.tensor.matmul(ps_q[:], lhs_h, wq_l, start=False, stop=False)
                        if ebg == 0:
                            nc.tensor.matmul(ps_w[t][:], lhs_h, wwh_s[:, c * H:(c + 1) * H],
                                             start=first, stop=False)
                            nc.tensor.matmul(ps_w[t][:], lhs_h, wwl_s[:, c * H:(c + 1) * H],
                                             start=False, stop=False)
                            nc.tensor.matmul(ps_w[t][:], lhs_l, wwh_s[:, c * H:(c + 1) * H],
                                             start=False, stop=False)
                            nc.tensor.matmul(ps_w[t][:], lhs_l, wwl_s[:, c * H:(c + 1) * H],
                                             start=False, stop=last)
                        nc.tensor.matmul(ps_q[:], lhs_l, wq_h, start=False, stop=last)
                    # evict this 4-head group, rope it, split, transpose
                    q32s = sb.tile([128, EBG], dt.float32, tag="q32", name=f"q32{t}{ebg}")
                    nc.scalar.copy(q32s[:], ps_q[:])
                    if ebg == 0:
                        t_w[t] = sb.tile([128, H], dt.float32, tag=f"w{t}", name=f"tw{t}")
                        nc.vector.tensor_scalar_mul(t_w[t][:], ps_w[t][:],
                                                    float((H * D) ** -0.5))
                    cosb = t_cos["cos" + t][:].rearrange("p (x m) -> p x m", x=1).to_broadcast([128, HPG, RD // 2])
                    sinb = t_cos["sin" + t][:].rearrange("p (x m) -> p x m", x=1).to_broadcast([128, HPG, RD // 2])
                    qv = q32s[:].rearrange("p (h d) -> p h d", h=HPG)
                    viewE = qv[:, :, D - RD::2]     # [128, HPG, 32] even rope cols
                    viewO = qv[:, :, D - RD + 1::2]
                    tmp = [sb.tile([128, HPG * (RD // 2)], dt.float32, tag=f"ropetmp{k}",
                                   name=f"ropetmp{t}{ebg}_{k}")
                           for k in range(4)]
                    tv = [x[:].rearrange("p (h m) -> p h m", h=HPG) for x in tmp]
                    nc.vector.tensor_mul(tv[0], viewO, sinb)  # tE
                    nc.vector.tensor_mul(tv[1], viewE, sinb)  # tO
                    nc.vector.tensor_mul(tv[2], viewE, cosb)  # m1
                    nc.vector.tensor_mul(tv[3], viewO, cosb)  # m2
                    nc.vector.tensor_sub(viewE, tv[2], tv[0])
                    nc.vector.tensor_add(viewO, tv[3], tv[1])
                    # split to fp16 pair
                    qh = sb.tile([128, EBG], dt.float16, tag="qh", name=f"qh{t}{ebg}")
                    ql = sb.tile([128, EBG], dt.float16, tag="ql", name=f"ql{t}{ebg}")
                    nc.vector.tensor_copy(qh[:], q32s[:])
                    nc.vector.tensor_sub(ql[:], q32s[:], qh[:])
                    # transpose 4 heads -> rqT [d, i] slices
                    for src, dst in ((qh, rqT[t][0]), (ql, rqT[t][1])):
                        ps_t = psq_pool.tile([128, EBG], dt.float16, tag="pstr",
                                             name=f"pstr{t}{ebg}")
                        for hh in range(HPG):
                            nc.tensor.transpose(ps_t[:, hh * D:(hh + 1) * D],
                                                src[:, hh * D:(hh + 1) * D], ident16[:])
                        nc.scalar.copy(dst[:, esl], ps_t[:])

        # ====== Phases S+T (per-core extents): scores, mask, topk ===========
        # Shared tiles at max shapes; arms use per-core-width views.
        Sacc = {"B": sb.tile([128, 2048], dt.float32, tag="saccB", name="SaccB"),
                "S": sb.tile([128, 1024], dt.float32, tag="saccS", name="SaccS")}
        vals = {t: sb.tile([128, TOPK], dt.float32, tag=f"vals{t}", name=f"vals{t}")
                for t in ("B", "S")}
        idxs = {t: sb.tile([128, TOPK], dt.uint32, tag=f"idx{t}", name=f"idx{t}")
                for t in ("B", "S")}
        cl = {t: sb.tile([128, TOPK], dt.float32, tag=f"cl{t}", name=f"cl{t}")
              for t in ("B", "S")}
        pss_pool = ctx.enter_context(tc.tile_pool(name="pss", bufs=2, space="PSUM"))
        ps_s = [pss_pool.tile([128, 2048], dt.float32, tag="pss", name=f"ps_s{i}")
                for i in range(2)]

        pid = nc.partition_id(engines=[ET.PE, ET.DVE, ET.SP])
        for core in tc.Switch(pid, NC):
            for t, blk in (("B", BLK_BIG[core]), ("S", BLK_SML[core])):
                e = _ext(blk)
                W = max(TOPK, e)          # topk working width
                R = _rounds(e)
                rqTh, rqTl = rqT[t]
                acc = Sacc[t]
                if e < W:
                    nc.vector.memset(acc[:, :W], 0.0)
                else:
                    nc.vector.memset(acc[:, :e], 0.0)
                njb = (e + 511) // 512
                for h in range(H):
                    ps = ps_s[h % 2]
                    lh = rqTh[:, h * D:(h + 1) * D]
                    ll = rqTl[:, h * D:(h + 1) * D]
                    # term-outer order: consecutive matmuls share the
                    # stationary lhs (fewer LDWEIGHTS)
                    for jb in range(njb):
                        sl = slice(jb * 512, min((jb + 1) * 512, e))
                        nc.tensor.matmul(ps[:, sl], lh, t_kTh[:, sl], start=True, stop=False)
                    for jb in range(njb):
                        sl = slice(jb * 512, min((jb + 1) * 512, e))
                        nc.tensor.matmul(ps[:, sl], lh, t_kTl[:, sl], start=False, stop=False)
                    for jb in range(njb):
                        sl = slice(jb * 512, min((jb + 1) * 512, e))
                        nc.tensor.matmul(ps[:, sl], ll, t_kTh[:, sl], start=False, stop=True)
                    nc.vector._custom_dve(ops["ANT_RELU_WACC"], out=acc[:, :e],
                                          in0=ps[:, :e], in1=acc[:, :e],
                                          s0=t_w[t][:, h:h + 1])
                # causal mask + sentinels (in place; pad cols e..W were memset
                # to 0 and are all masked since jrow >= e > irow there)
                nc.vector._custom_dve(ops["ANT_CAUSAL_SENT"], out=acc[:, :W],
                                      in0=acc[:, :W], in1=t_jrow[:, :W],
                                      s0=t_irow[t][:], s1=SENT_BASE)
                # extraction: per round find indices before destroying values
                for r in range(R):
                    v8 = vals[t][:, r * 8:(r + 1) * 8]
                    nc.vector.max(out=v8, in_=acc[:, :W])
                    nc.vector.max_index(out=idxs[t][:, r * 8:(r + 1) * 8],
                                        in_max=v8, in_values=acc[:, :W])
                    nc.vector.match_replace(out=acc[:, :W], in_to_replace=v8,
                                            in_values=acc[:, :W], imm_value=-3.0e38)
                nc.vector._custom_dve(ops["ANT_CLAMP_SENT"], out=cl[t][:, :R * 8],
                                      in0=vals[t][:, :R * 8], s0=CLAMP_AT, s1=-1.0e30)
                nc.sync.dma_start(outs[f"oV{t}"].ap()[:, :R * 8], cl[t][:, :R * 8])
                nc.sync.dma_start(outs[f"oI{t}"].ap()[:, :R * 8], idxs[t][:, :R * 8])

    nc.compile()
    _PROGRAM = nc
    return nc


# ---------------------------------------------------------------------------
# Host wrapper
# ---------------------------------------------------------------------------

def _host_inputs(hidden_states, cos, sin, wq, wk, ww):
    hid = hidden_states.reshape(S, HID).astype(np.float32)
    hT = np.ascontiguousarray(hid.T)
    hTh, hTl = _f16_pair(hT)
    wqh, wql = _f16_pair(wq.astype(np.float32))
    wkh, wkl = _f16_pair(wk.astype(np.float32))
    wwh, wwl = _f16_pair(ww.astype(np.float32))
    cosf = cos.reshape(S, RD // 2).astype(np.float32)
    sinf = sin.reshape(S, RD // 2).astype(np.float32)
    cos2 = np.repeat(cosf, 2, axis=1)            # [S, RD]
    sin2 = np.repeat(sinf, 2, axis=1)
    cos2T = np.ascontiguousarray(cos2.T)         # [RD, S]
    sin2T = np.ascontiguousarray(sin2.T)
    # rope rotation matrix: rot = M @ kvec on the last RD dims;
    # matmul computes lhsT.T @ rhs -> lhsT = M.T
    M = np.zeros((D, D), dtype=np.float32)
    for m in range(RD // 2):
        e = D - RD + 2 * m
        M[e, e + 1] = -1.0
        M[e + 1, e] = 1.0
    MT = np.ascontiguousarray(M.T)
    jrow = np.arange(S, dtype=np.float32).reshape(1, S)

    rep = {"hTh": hTh, "hTl": hTl, "wqh": wqh, "wql": wql, "wkh": wkh,
           "wkl": wkl, "wwh": wwh, "wwl": wwl, "cos2T": cos2T, "sin2T": sin2T,
           "MT": MT, "jrow": jrow}

    in_maps, row_maps = [], []
    for c in range(NC):
        rowsB = np.arange(128 * BLK_BIG[c], 128 * (BLK_BIG[c] + 1), dtype=np.int64)
        rowsS = np.arange(128 * BLK_SML[c], 128 * (BLK_SML[c] + 1), dtype=np.int64)
        own = np.concatenate([rowsB, rowsS])
        ohT = np.ascontiguousarray(hT[:, own])
        ohTh, ohTl = _f16_pair(ohT)
        m = dict(rep)
        m["ohTh"] = ohTh
        m["ohTl"] = ohTl
        m["cosB"] = np.ascontiguousarray(cosf[rowsB])
        m["sinB"] = np.ascontiguousarray(sinf[rowsB])
        m["cosS"] = np.ascontiguousarray(cosf[rowsS])
        m["sinS"] = np.ascontiguousarray(sinf[rowsS])
        m["irowB"] = rowsB.astype(np.float32).reshape(-1, 1)
        m["irowS"] = rowsS.astype(np.float32).reshape(-1, 1)
        in_maps.append(m)
        row_maps.append((rowsB, rowsS))
    return in_maps, row_maps


def kernel(hidden_states, cos, sin, wq, wk, ww, _trace=False):
    hidden_states = np.asarray(hidden_states)
    nc = _build_program()
    in_maps, row_maps = _host_inputs(np.asarray(hidden_states), np.asarray(cos),
                                     np.asarray(sin), np.asarray(wq), np.asarray(wk),
                                     np.asarray(ww))
    res = bass_utils.run_bass_kernel_spmd(nc, in_maps, core_ids=list(range(NC)),
                                          trace=_trace)
    scores = np.zeros((B, S, TOPK), dtype=np.float32)
    idxs = np.zeros((B, S, TOPK), dtype=np.int32)
    # deterministic sentinel tail beyond the produced ranks (rows with
    # extent < 512): rank r > i has val -1e30 and idx r
    tail_idx = np.arange(TOPK, dtype=np.int32)
    for c in range(NC):
        rowsB, rowsS = row_maps[c]
        r = res.results[c]
        for t, rows in (("B", rowsB), ("S", rowsS)):
            e = _ext(BLK_BIG[c] if t == "B" else BLK_SML[c])
            n = _rounds(e) * 8
            scores[0, rows, :n] = r[f"oV{t}"][:, :n]
            idxs[0, rows, :n] = r[f"oI{t}"][:, :n].astype(np.int32)
            if n < TOPK:
                scores[0, rows, n:] = -1.0e30
                idxs[0, rows, n:] = tail_idx[n:]
    kernel._last_result = res
    return scores, idxs
